# revision 30
# baseline (speedup 1.0000x reference)
"""Fused multi-head attention block (QKV proj + softmax attention + out-proj
+ LayerNorm) for Trainium2, sharded over 8 NeuronCores.

Sharding: tensor-parallel over heads. Core c owns heads [4c, 4c+4).

Structure (per core):
  - QKV projection for its 4 heads over all 4096 rows (both batches), K/Q in
    transposed [dim, rows] layout, V transposed to [key, ch] tiles.
  - Attention per 512-wide q-block: per 128-key tile, 4 row-packed QK^T
    matmuls land in a 4-bank PSUM tile; ONE 2048-elem ACT exp converts it to
    a bf16 SBUF tile; 4 col-packed PV matmuls + 4 ones-matmuls accumulate
    the numerator/denominator in PSUM across key tiles.
  - The softmax denominator is inverted on DVE, broadcast across the head's
    32 channels with a tiny indicator matmul, and fused into the PSUM->SBUF
    drain, so the AllToAll payload is already-normalized attention output.
  - One AllToAll per q-block (8 total) reshards head-parallel -> row-parallel
    while later q-blocks compute; each core then runs the 1024x1024 output
    projection + bias + LayerNorm for its 64 rows of that q-block.
  - LayerNorm rstd = exp(-0.5*ln(var+eps)) keeps ACT on a single function
    table (no Exp<->Sqrt table reloads mid-attention).

dtypes: fp32 storage; QKV and S^T matmuls in float32r; exp/PV/out-proj bf16.
"""
import sys

for _p in ("/opt/trn_rl_repo", "/root/.axon_site/_ro/trn_rl_repo"):
    if _p not in sys.path:
        sys.path.insert(0, _p)

import numpy as np

import concourse.bass as bass
import concourse.tile as tile
from concourse import bacc, mybir
from concourse.masks import make_identity

F32 = mybir.dt.float32
F32R = mybir.dt.float32r
BF16 = mybir.dt.bfloat16
AF = mybir.ActivationFunctionType
ALU = mybir.AluOpType

N_CORES = 8
B, N, DIM = 2, 2048, 1024
HEADS, DH = 32, 32           # 32 heads x 32 dim/head
HPC = HEADS // N_CORES       # 4 heads per core
ROWS = B * N                 # 4096 global rows
SCALE = DH ** -0.5
EPS = 1e-6
KT = N // 128                # 16 key tiles per batch
QB = 512                     # q-block width
NQB = N // QB                # 4 q-blocks per batch
NU = B * NQB                 # 8 exchange units (one per q-block)
RPU = QB // N_CORES          # 64 rows per core per unit
RC = 256                     # projection row-chunk
NRC = ROWS // RC             # 16 row chunks


def _build(debug=False):
    nc = bacc.Bacc("TRN2", target_bir_lowering=False, debug=False,
                   num_devices=N_CORES)

    xT_d = nc.dram_tensor("xT", [DIM, ROWS], F32R, kind="ExternalInput").ap()
    wqkv_d = nc.dram_tensor("wqkv", [DIM, 3 * HPC * DH], F32R,
                            kind="ExternalInput").ap()
    wout_d = nc.dram_tensor("wout", [DIM, DIM], F32, kind="ExternalInput").ap()
    bout_d = nc.dram_tensor("bout", [DIM], F32, kind="ExternalInput").ap()
    gamma_d = nc.dram_tensor("gamma", [DIM], F32, kind="ExternalInput").ap()
    beta_d = nc.dram_tensor("beta", [DIM], F32, kind="ExternalInput").ap()
    out_d = nc.dram_tensor("out", [NU * RPU, DIM], F32,
                           kind="ExternalOutput").ap()
    if debug:
        dbg_qT = nc.dram_tensor("dbg_qT", [128, ROWS], F32,
                                kind="ExternalOutput").ap()
        dbg_kT = nc.dram_tensor("dbg_kT", [128, ROWS], F32,
                                kind="ExternalOutput").ap()
        dbg_V = nc.dram_tensor("dbg_V", [128, B * KT, 128], BF16,
                               kind="ExternalOutput").ap()
        dbg_att = nc.dram_tensor("dbg_att", [NU, 128, QB], BF16,
                                 kind="ExternalOutput").ap()
        dbg_rec = nc.dram_tensor("dbg_rec", [NU, 128, QB], F32,
                                 kind="ExternalOutput").ap()
        dbg_asb = nc.dram_tensor("dbg_asb", [NU, 128, N_CORES, RPU], BF16,
                                 kind="ExternalOutput").ap()
        dbg_osb = nc.dram_tensor("dbg_osb", [NU, RPU, DIM], F32,
                                 kind="ExternalOutput").ap()

    with tile.TileContext(nc) as tc:
        with (
            tc.tile_pool(name="const", bufs=1) as const,
            tc.tile_pool(name="work", bufs=1) as work,
            tc.tile_pool(name="ps", bufs=1, space="PSUM") as ps,
            tc.tile_pool(name="dram", bufs=1, space="DRAM") as dram,
        ):
            # ---------------- constants / weights ----------------
            wqkv_sb = const.tile([128, 8, 3 * HPC * DH], F32R)
            nc.sync.dma_start(
                wqkv_sb[:], wqkv_d.rearrange("(kc p) m -> p kc m", p=128))
            ones_bf = const.tile([128, 1], BF16)
            nc.vector.memset(ones_bf[:], 1.0)
            ident = const.tile([128, 128], F32)
            make_identity(nc, ident[:])
            eps_sb = const.tile([128, 1], F32)
            nc.vector.memset(eps_sb[:], EPS)
            # indicator for denominator broadcast: E[32h, 32h:32h+32] = 1.
            # Single-row memsets at partitions 32/64/96 are not supported, so
            # build E^T (32-partition-aligned column blocks) and PE-transpose.
            e4t = const.tile([128, 128], F32)
            nc.vector.memset(e4t[:], 0.0)
            for h in range(HPC):
                nc.vector.memset(
                    e4t[32 * h:32 * h + 32, 32 * h:32 * h + 1], 1.0)
            e4 = const.tile([128, 128], BF16)
            tpE = ps.tile([128, 128], F32, tag="aux", name="tpE")
            nc.tensor.transpose(tpE[:], e4t[:], ident[:])
            nc.vector.tensor_copy(e4[:], tpE[:])

            # row-broadcast vectors [128, 1024]
            bout_bc = const.tile([128, DIM], F32)
            nc.gpsimd.dma_start(out=bout_bc[:], in_=bass.AP(
                tensor=bout_d.tensor, offset=bout_d.offset,
                ap=[[0, 128], [1, DIM]]))
            gamma_bc = const.tile([128, DIM], F32)
            nc.gpsimd.dma_start(out=gamma_bc[:], in_=bass.AP(
                tensor=gamma_d.tensor, offset=gamma_d.offset,
                ap=[[0, 128], [1, DIM]]))
            beta_bc = const.tile([128, DIM], F32)
            nc.gpsimd.dma_start(out=beta_bc[:], in_=bass.AP(
                tensor=beta_d.tensor, offset=beta_d.offset,
                ap=[[0, 128], [1, DIM]]))
            # w_out -> bf16 [128, 8, 1024]
            wout_bf = const.tile([128, 8, DIM], BF16)

            # ---------------- persistent activations ----------------
            qT_sb = const.tile([128, ROWS], F32R)   # 4h x 32d on partitions
            kT_sb = const.tile([128, ROWS], F32R)
            V_sb = const.tile([128, B * KT, 128], BF16)  # [key%128, kt, ch]

            # ---------------- dram bounce buffers ----------------
            a2a_in = [dram.tile([N_CORES, 128, RPU], BF16, name=f"a2ai_{u}")
                      for u in range(NU)]
            a2a_out = [dram.tile([N_CORES, 128, RPU], BF16, name=f"a2ao_{u}")
                       for u in range(NU)]

            # ---------------- phase A: projections ----------------
            xt_tiles = {}

            def proj_dma(rc, queue=0):
                xt = work.tile([128, 8, RC], F32R, tag="xt", bufs=4,
                               name=f"xt_{rc}")
                eng = nc.sync if queue == 0 else nc.gpsimd
                eng.dma_start(
                    xt[:],
                    xT_d[:, rc * RC:(rc + 1) * RC]
                    .rearrange("(kc p) n -> p kc n", p=128))
                xt_tiles[rc] = xt

            def proj_compute(rc):
                xt = xt_tiles.pop(rc)
                for name, mofs, dst in (("q", 0, qT_sb), ("k", 128, kT_sb)):
                    pp = ps.tile([128, RC], F32, tag="aux",
                                 name=f"pp_{name}_{rc}")
                    for kc in range(8):
                        nc.tensor.matmul(
                            pp[:], wqkv_sb[:, kc, mofs:mofs + 128],
                            xt[:, kc, :], start=(kc == 0), stop=(kc == 7))
                    nc.vector.tensor_copy(dst[:, rc * RC:(rc + 1) * RC], pp[:])
                # v: project (vT layout), cast bf16, PE-transpose into V_sb
                pv_ = ps.tile([128, RC], F32, tag="aux", name=f"pp_v_{rc}")
                for kc in range(8):
                    nc.tensor.matmul(
                        pv_[:], wqkv_sb[:, kc, 256:384], xt[:, kc, :],
                        start=(kc == 0), stop=(kc == 7))
                vt = work.tile([128, RC], F32, tag="vt", bufs=2,
                               name=f"vt_{rc}")
                nc.vector.tensor_copy(vt[:], pv_[:])
                for i in range(RC // 128):
                    tp = ps.tile([128, 128], F32, tag="aux",
                                 name=f"tp_{rc}_{i}")
                    nc.tensor.transpose(
                        tp[:], vt[:, i * 128:(i + 1) * 128], ident[:])
                    nc.vector.tensor_copy(
                        V_sb[:, rc * (RC // 128) + i, :], tp[:])

            wout_stage = {}

            def wout_dma(j):
                st = work.tile([128, DIM], F32, tag="wstage", bufs=3,
                               name=f"wst_{j}")
                nc.gpsimd.dma_start(st[:], wout_d[j * 128:(j + 1) * 128, :])
                wout_stage[j] = st

            def wout_cast(j):
                nc.vector.tensor_copy(wout_bf[:, j, :], wout_stage.pop(j))

            # ---------------- phase B: attention q-block ----------------
            DEN_LAG = 4

            class Window:
                """One 512-wide q-block: QK->exp->PV/den pipeline state."""

                def __init__(self, w, b, qb):
                    self.w, self.b, self.qb = w, b, qb
                    self.qsl = qT_sb[:, b * N + qb * QB: b * N + qb * QB + QB]
                    self.pvp = ps.tile([128, QB], F32, tag="pvt", bufs=2,
                                       name=f"pv_{w}")
                    self.dnp = ps.tile([128, QB], F32, tag="dn",
                                       name=f"dn_{w}")
                    self.exps = []

                def qk_exp(self, kt):
                    # QK head-pairs interleave with two half-exp ACT instrs
                    # so the PSUM-bank WAR never leaves ACT waiting on QK
                    b = self.b
                    ksl = kT_sb[:, b * N + kt * 128: b * N + kt * 128 + 128]
                    qk = ps.tile([128, HPC, QB], F32, tag="qk",
                                 name=f"qk_{self.w}_{kt}")
                    e = work.tile([128, HPC, QB], BF16, tag="exp",
                                  bufs=DEN_LAG + 2, name=f"exp_{self.w}_{kt}")
                    for hp in range(2):
                        for h in (2 * hp, 2 * hp + 1):
                            nc.tensor.matmul(
                                qk[:, h, :],
                                ksl[32 * h:32 * h + 32, :],
                                self.qsl[32 * h:32 * h + 32, :],
                                start=True, stop=True,
                                tile_position=(32 * h, 0))
                        nc.scalar.activation(
                            e[:, 2 * hp:2 * hp + 2, :],
                            qk[:, 2 * hp:2 * hp + 2, :], AF.Exp, scale=SCALE)
                    self.exps.append(e)

                def pv(self, kt):
                    e = self.exps[kt]
                    for h in range(HPC):
                        nc.tensor.matmul(
                            self.pvp[32 * h:32 * h + 32, :],
                            V_sb[:, self.b * KT + kt, 32 * h:32 * h + 32],
                            e[:, h, :], start=(kt == 0), stop=(kt == KT - 1),
                            tile_position=(0, 32 * h))

                def den(self, kt):
                    e = self.exps[kt]
                    for h in range(HPC):
                        nc.tensor.matmul(
                            self.dnp[32 * h:32 * h + 1, :],
                            ones_bf[:], e[:, h, :],
                            start=(kt == 0), stop=(kt == KT - 1),
                            tile_position=(0, 32 * h))

            def epilogue(win):
                """Normalize pvp by recip(den) broadcast over each head's 32
                channels (indicator matmul), drain to bf16, exchange."""
                w = win.w
                den4 = work.tile([128, QB], F32, tag="den4", bufs=2,
                                 name=f"den4_{w}")
                nc.vector.memset(den4[:], 1.0)
                for h in range(HPC):
                    nc.vector.tensor_copy(den4[32 * h:32 * h + 1, :],
                                          win.dnp[32 * h:32 * h + 1, :])
                recipf = work.tile([128, QB], F32, tag="recipf", bufs=2,
                                   name=f"recipf_{w}")
                nc.vector.reciprocal_approx_fast(out=recipf[:], in_=den4[:])
                recipb = work.tile([128, QB], BF16, tag="recipb", bufs=2,
                                   name=f"recipb_{w}")
                nc.vector.tensor_copy(recipb[:], recipf[:])
                rbc = ps.tile([128, QB], F32, tag="aux", name=f"rbc_{w}")
                nc.tensor.matmul(rbc[:], e4[:], recipb[:],
                                 start=True, stop=True)
                rbc_sb = work.tile([128, QB], F32, tag="rbcsb", bufs=2,
                                   name=f"rbcsb_{w}")
                nc.vector.tensor_copy(rbc_sb[:], rbc[:])
                att = work.tile([128, QB], BF16, tag="att", bufs=2,
                                name=f"att_{w}")
                nc.vector.tensor_tensor(att[:], win.pvp[:], rbc_sb[:],
                                        ALU.mult)
                if debug:
                    nc.sync.dma_start(dbg_att[w], att[:])
                    nc.sync.dma_start(dbg_rec[w], rbc_sb[:])
                for j in range(N_CORES):
                    nc.sync.dma_start(a2a_in[w][j],
                                      att[:, j * RPU:(j + 1) * RPU])
                nc.gpsimd.collective_compute(
                    "AllToAll", ALU.bypass,
                    replica_groups=[list(range(N_CORES))],
                    ins=[a2a_in[w].opt()], outs=[a2a_out[w].opt()])

            # ---------------- phase C: out-proj + LN per unit ----------
            asb_tiles = {}

            def outproj_load(u):
                asb = work.tile([128, N_CORES, RPU], BF16, tag="a2asb",
                                bufs=2, name=f"asb_{u}")
                for j in range(N_CORES):
                    nc.sync.dma_start(asb[:, j, :], a2a_out[u][j])
                asb_tiles[u] = asb

            def outproj_unit(u):
                asb = asb_tiles.pop(u)
                if debug:
                    nc.sync.dma_start(dbg_asb[u], asb[:])
                osb = work.tile([RPU, DIM], F32, tag="osb", bufs=2,
                                name=f"osb_{u}")
                for nb in range(2):
                    op = ps.tile([RPU, 512], F32, tag="aux",
                                 name=f"op_{u}_{nb}")
                    for j in range(N_CORES):
                        nc.tensor.matmul(
                            op[:], asb[:, j, :],
                            wout_bf[:, j, nb * 512:(nb + 1) * 512],
                            start=(j == 0), stop=(j == N_CORES - 1))
                    nc.vector.tensor_tensor(
                        osb[:, nb * 512:(nb + 1) * 512], op[:],
                        bout_bc[0:RPU, nb * 512:(nb + 1) * 512], ALU.add)
                if debug:
                    nc.sync.dma_start(dbg_osb[u], osb[:])
                # LayerNorm over the 1024 free dim
                stats = work.tile([RPU, 2, 6], F32, tag="stats", bufs=2,
                                  name=f"stats_{u}")
                for sg in range(2):
                    nc.vector.bn_stats(out=stats[:, sg, :],
                                       in_=osb[:, sg * 512:(sg + 1) * 512])
                mv = work.tile([RPU, 2], F32, tag="mv", bufs=2,
                               name=f"mv_{u}")
                nc.vector.bn_aggr(out=mv[:], in_=stats[:])
                # rstd = exp(-0.5*ln(var+eps)); Ln+Exp share one ACT table
                lnv = work.tile([RPU, 1], F32, tag="lnv", bufs=2,
                                name=f"lnv_{u}")
                nc.scalar.activation(out=lnv[:], in_=mv[:, 1:2], func=AF.Ln,
                                     bias=eps_sb[0:RPU, :], scale=1.0)
                rstd = work.tile([RPU, 1], F32, tag="rstd", bufs=2,
                                 name=f"rstd_{u}")
                nc.scalar.activation(out=rstd[:], in_=lnv[:], func=AF.Exp,
                                     bias=0.0, scale=-0.5)
                nc.vector.tensor_scalar(
                    out=osb[:], in0=osb[:], scalar1=mv[:, 0:1],
                    scalar2=rstd[:], op0=ALU.subtract, op1=ALU.mult)
                nc.vector.tensor_tensor(osb[:], osb[:], gamma_bc[0:RPU, :],
                                        ALU.mult)
                nc.vector.tensor_tensor(osb[:], osb[:], beta_bc[0:RPU, :],
                                        ALU.add)
                nc.sync.dma_start(out_d[u * RPU:(u + 1) * RPU, :], osb[:])

            # ---------------- issue order (software pipeline) ----------
            # Window w's kt loop carries: the epilogue of window w-1 (at
            # kt=2, so its DVE chain overlaps QK/exp instead of blocking the
            # PE FIFO at the boundary), out-proj of unit w-2 (DMAs at kt=6,
            # matmuls at kt=10, when its AllToAll is long done), and
            # projection/weight-load chunks at fixed slots with their DMAs
            # prefetched ~4 key-tiles ahead, so no PE-FIFO entry ever waits
            # on a freshly issued DMA.
            proj_dma(0, 0)
            proj_dma(1, 1)
            proj_dma(2, 0)
            proj_dma(3, 1)
            proj_compute(0)
            proj_compute(1)
            # remaining batch-0 chunks inside window 0; batch-1 chunks and
            # w_out loads spread over windows 1-3
            pc_slots = {0: {0: 2, 2: 3, 4: 4, 6: 5, 8: 6, 10: 7},
                        1: {5: 8, 9: 9, 13: 10},
                        2: {5: 11, 9: 12, 13: 13},
                        3: {5: 14, 11: 15}}
            pd_slots = {0: {0: 4, 2: 5, 4: 6, 6: 7},
                        1: {1: 8, 5: 9, 9: 10},
                        2: {1: 11, 5: 12, 9: 13},
                        3: {1: 14, 7: 15}}
            wd_slots = {1: {2: 0, 4: 1, 6: 2, 8: 3, 10: 4, 12: 5, 14: 6},
                        2: {0: 7}}
            wc_slots = {1: {8: 0, 10: 1, 12: 2, 14: 3},
                        2: {1: 4, 3: 5, 5: 6, 7: 7}}

            prev = None
            for w in range(NU):
                b, qb = divmod(w, NQB)
                win = Window(w, b, qb)
                for kt in range(KT):
                    win.qk_exp(kt)
                    if kt == 2 and prev is not None:
                        epilogue(prev)
                    if kt >= 1:
                        win.pv(kt - 1)
                    if kt >= DEN_LAG:
                        win.den(kt - DEN_LAG)
                    rc = pd_slots.get(w, {}).get(kt)
                    if rc is not None:
                        proj_dma(rc, rc % 2)
                    rc = pc_slots.get(w, {}).get(kt)
                    if rc is not None:
                        proj_compute(rc)
                    j = wd_slots.get(w, {}).get(kt)
                    if j is not None:
                        wout_dma(j)
                    j = wc_slots.get(w, {}).get(kt)
                    if j is not None:
                        wout_cast(j)
                    if kt == 6 and w >= 2:
                        outproj_load(w - 2)
                    if kt == 10 and w >= 2:
                        outproj_unit(w - 2)
                win.pv(KT - 1)
                for kt in range(KT - DEN_LAG, KT):
                    win.den(kt)
                prev = win
            epilogue(prev)
            outproj_load(NU - 2)
            outproj_unit(NU - 2)
            outproj_load(NU - 1)
            outproj_unit(NU - 1)
            if debug:
                nc.sync.dma_start(dbg_qT, qT_sb[:].bitcast(F32))
                nc.sync.dma_start(dbg_kT, kT_sb[:].bitcast(F32))
                nc.sync.dma_start(dbg_V, V_sb[:])

    nc.compile()
    return nc


class _Runner:
    """Compile once; run the SPMD kernel on 8 cores via PJRT repeatedly."""

    def __init__(self, debug=False):
        self.nc = _build(debug=debug)
        import jax
        from jax.sharding import Mesh, NamedSharding, PartitionSpec
        from jax.experimental.shard_map import shard_map
        from concourse import bass2jax
        bass2jax.install_neuronx_cc_hook()

        nc = self.nc
        part_name = (nc.partition_id_tensor.name
                     if nc.partition_id_tensor else None)
        in_names, out_names, out_avals = [], [], []
        for alloc in nc.m.functions[0].allocations:
            if not isinstance(alloc, mybir.MemoryLocationSet):
                continue
            name = alloc.memorylocations[0].name
            if alloc.kind == "ExternalInput":
                if name != part_name:
                    in_names.append(name)
            elif alloc.kind == "ExternalOutput":
                out_names.append(name)
                out_avals.append(jax.core.ShapedArray(
                    tuple(alloc.tensor_shape), mybir.dt.np(alloc.dtype)))
        self.in_names = list(in_names)
        self.out_names = out_names
        self.out_avals = out_avals
        all_in_names = in_names + out_names
        if part_name is not None:
            all_in_names = all_in_names + [part_name]

        def _body(*args):
            operands = list(args)
            if part_name is not None:
                operands.append(bass2jax.partition_id_tensor())
            outs = bass2jax._bass_exec_p.bind(
                *operands, out_avals=tuple(out_avals),
                in_names=tuple(all_in_names), out_names=tuple(out_names),
                lowering_input_output_aliases=(),
                sim_require_finite=True, sim_require_nnan=True, nc=nc)
            return tuple(outs)

        devices = jax.devices()[:N_CORES]
        mesh = Mesh(np.asarray(devices), ("core",))
        self.sharding = NamedSharding(mesh, PartitionSpec("core"))
        nin = len(self.in_names) + len(out_names)
        self.fn = jax.jit(shard_map(
            _body, mesh=mesh, in_specs=(PartitionSpec("core"),) * nin,
            out_specs=(PartitionSpec("core"),) * len(out_names),
            check_rep=False))
        self.jax = jax

    def stage(self, in_maps):
        """Concatenate per-core inputs + zero outputs; device_put SHARDED so
        no per-call resharding happens."""
        concat = [np.concatenate([m[name] for m in in_maps], axis=0)
                  for name in self.in_names]
        zeros = [np.zeros((N_CORES * a.shape[0], *a.shape[1:]), a.dtype)
                 for a in self.out_avals]
        return [self.jax.device_put(x, self.sharding)
                for x in concat + zeros]

    def run_staged(self, staged):
        outs = self.fn(*staged)
        self.jax.block_until_ready(outs)
        return outs

    def run(self, in_maps):
        outs = self.run_staged(self.stage(in_maps))
        return [
            {name: np.asarray(outs[i]).reshape(
                N_CORES, *self.out_avals[i].shape)[c]
             for i, name in enumerate(self.out_names)}
            for c in range(N_CORES)
        ]


_RUNNER = None


def _get_runner():
    global _RUNNER
    if _RUNNER is None:
        _RUNNER = _Runner()
    return _RUNNER


def _make_in_maps(x, w_qkv, w_out, b_out, ln_gamma, ln_beta):
    x = np.asarray(x, dtype=np.float32)
    w_qkv = np.asarray(w_qkv, dtype=np.float32)
    w_out = np.asarray(w_out, dtype=np.float32)
    b_out = np.asarray(b_out, dtype=np.float32)
    ln_gamma = np.asarray(ln_gamma, dtype=np.float32)
    ln_beta = np.asarray(ln_beta, dtype=np.float32)

    xT = np.ascontiguousarray(x.reshape(ROWS, DIM).T)
    in_maps = []
    for c in range(N_CORES):
        h0 = HPC * c * DH
        cols = np.concatenate([
            w_qkv[:, h0:h0 + HPC * DH],
            w_qkv[:, DIM + h0:DIM + h0 + HPC * DH],
            w_qkv[:, 2 * DIM + h0:2 * DIM + h0 + HPC * DH],
        ], axis=1)
        in_maps.append({
            "xT": xT,
            "wqkv": np.ascontiguousarray(cols),
            "wout": w_out,
            "bout": b_out,
            "gamma": ln_gamma,
            "beta": ln_beta,
        })
    return in_maps


def kernel(x, w_qkv, w_out, b_out, ln_gamma, ln_beta):
    runner = _get_runner()
    in_maps = _make_in_maps(x, w_qkv, w_out, b_out, ln_gamma, ln_beta)
    results = runner.run(in_maps)
    # core c, unit u=(b, qb) holds rows b*2048 + qb*512 + 64*c + [0..64)
    out = np.empty((ROWS, DIM), np.float32)
    for c in range(N_CORES):
        oc = results[c]["out"]
        for u in range(NU):
            b, qb = divmod(u, NQB)
            r0 = b * N + qb * QB + RPU * c
            out[r0:r0 + RPU] = oc[u * RPU:(u + 1) * RPU]
    return out.reshape(B, N, DIM).astype(np.float32)


# revision 31
# speedup vs baseline: 1.3021x; 1.3021x over previous
"""Fused multi-head attention block (QKV proj + softmax attention + out-proj
+ LayerNorm) for Trainium2, sharded over 8 NeuronCores.

Sharding: tensor-parallel over heads. Core c owns heads [4c, 4c+4).

Structure (per core):
  - QKV projection for its 4 heads over all 4096 rows (both batches), K/Q in
    transposed [dim, rows] layout, V transposed to [key, ch] tiles.
  - Attention per 512-wide q-block: per 128-key tile, 4 row-packed QK^T
    matmuls land in two 2-bank PSUM tiles; two 1024-elem ACT exps convert
    them to bf16 SBUF tiles (two independent QK->exp chains, so ACT runs
    back-to-back while the other chain's QK writes). PV uses [V_h | ones]
    33-column stationaries, 2-head column packing, so the softmax
    denominator accumulates as a free 33rd output row.
  - The softmax denominator is inverted on DVE, broadcast across the head's
    32 channels with a tiny indicator matmul, and fused into the PSUM->SBUF
    drain, so the AllToAll payload is already-normalized attention output.
  - One AllToAll per q-block (8 total) reshards head-parallel -> row-parallel
    while later q-blocks compute; each core then runs the 1024x1024 output
    projection + bias + LayerNorm for its 64 rows of that q-block.
  - LayerNorm rstd = exp(-0.5*ln(var+eps)) keeps ACT on a single function
    table (no Exp<->Sqrt table reloads mid-attention).

dtypes: fp32 storage; QKV and S^T matmuls in float32r; exp/PV/out-proj bf16.
"""
import sys

for _p in ("/opt/trn_rl_repo", "/root/.axon_site/_ro/trn_rl_repo"):
    if _p not in sys.path:
        sys.path.insert(0, _p)

import numpy as np

import concourse.bass as bass
import concourse.tile as tile
from concourse import bacc, mybir
from concourse.masks import make_identity

F32 = mybir.dt.float32
F32R = mybir.dt.float32r
BF16 = mybir.dt.bfloat16
AF = mybir.ActivationFunctionType
ALU = mybir.AluOpType

N_CORES = 8
B, N, DIM = 2, 2048, 1024
HEADS, DH = 32, 32           # 32 heads x 32 dim/head
HPC = HEADS // N_CORES       # 4 heads per core
ROWS = B * N                 # 4096 global rows
SCALE = DH ** -0.5
EPS = 1e-6
KT = N // 128                # 16 key tiles per batch
QB = 512                     # q-block width
NQB = N // QB                # 4 q-blocks per batch
NU = B * NQB                 # 8 exchange units (one per q-block)
RPU = QB // N_CORES          # 64 rows per core per unit
RC = 512                     # projection row-chunk
NRC = ROWS // RC             # 8 row chunks


def _build(debug=False):
    nc = bacc.Bacc("TRN2", target_bir_lowering=False, debug=False,
                   num_devices=N_CORES)

    xT_d = nc.dram_tensor("xT", [DIM, ROWS], F32R, kind="ExternalInput").ap()
    wqkv_d = nc.dram_tensor("wqkv", [DIM, 3 * HPC * DH], F32R,
                            kind="ExternalInput").ap()
    wout_d = nc.dram_tensor("wout", [DIM, DIM], F32, kind="ExternalInput").ap()
    bout_d = nc.dram_tensor("bout", [DIM], F32, kind="ExternalInput").ap()
    gamma_d = nc.dram_tensor("gamma", [DIM], F32, kind="ExternalInput").ap()
    beta_d = nc.dram_tensor("beta", [DIM], F32, kind="ExternalInput").ap()
    out_d = nc.dram_tensor("out", [NU * RPU, DIM], F32,
                           kind="ExternalOutput").ap()
    if debug:
        dbg_qT = nc.dram_tensor("dbg_qT", [128, ROWS], F32,
                                kind="ExternalOutput").ap()
        dbg_kT = nc.dram_tensor("dbg_kT", [128, ROWS], F32,
                                kind="ExternalOutput").ap()
        dbg_V = nc.dram_tensor("dbg_V", [128, B * KT, HPC, 33], BF16,
                               kind="ExternalOutput").ap()
        dbg_att = nc.dram_tensor("dbg_att", [NU, 128, QB], BF16,
                                 kind="ExternalOutput").ap()
        dbg_rec = nc.dram_tensor("dbg_rec", [NU, 128, QB], F32,
                                 kind="ExternalOutput").ap()
        dbg_asb = nc.dram_tensor("dbg_asb", [NU, 128, N_CORES, RPU], BF16,
                                 kind="ExternalOutput").ap()
        dbg_osb = nc.dram_tensor("dbg_osb", [NU, RPU, DIM], F32,
                                 kind="ExternalOutput").ap()

    with tile.TileContext(nc) as tc:
        with (
            tc.tile_pool(name="const", bufs=1) as const,
            tc.tile_pool(name="work", bufs=1) as work,
            tc.tile_pool(name="ps", bufs=1, space="PSUM") as ps,
            tc.tile_pool(name="dram", bufs=1, space="DRAM") as dram,
        ):
            # ---------------- constants / weights ----------------
            wqkv_sb = const.tile([128, 8, 3 * HPC * DH], F32R)
            nc.sync.dma_start(
                wqkv_sb[:], wqkv_d.rearrange("(kc p) m -> p kc m", p=128))
            ident = const.tile([128, 128], F32)
            make_identity(nc, ident[:])
            eps_sb = const.tile([128, 1], F32)
            nc.vector.memset(eps_sb[:], EPS)
            # indicator for denominator broadcast: E[32h, 32h:32h+32] = 1.
            # Single-row memsets at partitions 32/64/96 are not supported, so
            # build E^T (32-partition-aligned column blocks) and PE-transpose.
            e4t = const.tile([128, 128], F32)
            nc.vector.memset(e4t[:], 0.0)
            for h in range(HPC):
                nc.vector.memset(
                    e4t[32 * h:32 * h + 32, 32 * h:32 * h + 1], 1.0)
            e4 = const.tile([128, 128], BF16)
            tpE = ps.tile([128, 128], F32, tag="aux", name="tpE")
            nc.tensor.transpose(tpE[:], e4t[:], ident[:])
            nc.vector.tensor_copy(e4[:], tpE[:])
            # row-broadcast vectors [128, 1024]
            bout_bc = const.tile([128, DIM], F32)
            nc.gpsimd.dma_start(out=bout_bc[:], in_=bass.AP(
                tensor=bout_d.tensor, offset=bout_d.offset,
                ap=[[0, 128], [1, DIM]]))
            gamma_bc = const.tile([128, DIM], F32)
            nc.gpsimd.dma_start(out=gamma_bc[:], in_=bass.AP(
                tensor=gamma_d.tensor, offset=gamma_d.offset,
                ap=[[0, 128], [1, DIM]]))
            beta_bc = const.tile([128, DIM], F32)
            nc.gpsimd.dma_start(out=beta_bc[:], in_=bass.AP(
                tensor=beta_d.tensor, offset=beta_d.offset,
                ap=[[0, 128], [1, DIM]]))
            # w_out -> bf16 [128, 8, 1024]
            wout_bf = const.tile([128, 8, DIM], BF16)

            # ---------------- persistent activations ----------------
            qT_sb = const.tile([128, ROWS], F32R)   # 4h x 32d on partitions
            kT_sb = const.tile([128, ROWS], F32R)
            # V in [key, head, ch|ones] layout: column 32 of each head's 33
            # is 1.0 so PV's packed matmul also produces the denominator
            V_sb = const.tile([128, B * KT, HPC, 33], BF16)
            nc.vector.memset(V_sb[:], 1.0)

            # ---------------- dram bounce buffers ----------------
            a2a_in = [dram.tile([N_CORES, 128, RPU], BF16, name=f"a2ai_{u}")
                      for u in range(NU)]
            a2a_out = [dram.tile([N_CORES, 128, RPU], BF16, name=f"a2ao_{u}")
                       for u in range(NU)]

            # ---------------- phase A: projections ----------------
            xt_tiles = {}

            def proj_dma(rc):
                xt = work.tile([128, 8, RC], F32R, tag="xt", bufs=2,
                               name=f"xt_{rc}")
                half = RC // 2
                src = xT_d[:, rc * RC:(rc + 1) * RC]
                nc.sync.dma_start(
                    xt[:, :, 0:half],
                    src[:, 0:half].rearrange("(kc p) n -> p kc n", p=128))
                nc.gpsimd.dma_start(
                    xt[:, :, half:RC],
                    src[:, half:RC].rearrange("(kc p) n -> p kc n", p=128))
                xt_tiles[rc] = xt

            def proj_part(rc, which):
                xt = xt_tiles[rc]
                if which == "q" or which == "k":
                    mofs = 0 if which == "q" else 128
                    dst = qT_sb if which == "q" else kT_sb
                    pp = ps.tile([128, RC], F32, tag="aux",
                                 name=f"pp_{which}_{rc}")
                    for kc in range(8):
                        nc.tensor.matmul(
                            pp[:], wqkv_sb[:, kc, mofs:mofs + 128],
                            xt[:, kc, :], start=(kc == 0), stop=(kc == 7))
                    nc.vector.tensor_copy(dst[:, rc * RC:(rc + 1) * RC], pp[:])
                    if which == "q":
                        xt_tiles.pop(rc)   # q is issued last per chunk
                    return
                # v: project (vT layout), PE-transpose, scatter into the
                # [key, head, ch] slots (column 32 stays 1.0)
                pv_ = ps.tile([128, RC], F32, tag="aux", name=f"pp_v_{rc}")
                for kc in range(8):
                    nc.tensor.matmul(
                        pv_[:], wqkv_sb[:, kc, 256:384], xt[:, kc, :],
                        start=(kc == 0), stop=(kc == 7))
                vt = work.tile([128, RC], F32, tag="vt", bufs=2,
                               name=f"vt_{rc}")
                nc.vector.tensor_copy(vt[:], pv_[:])
                for i in range(RC // 128):
                    tp = ps.tile([128, 128], F32, tag="aux",
                                 name=f"tp_{rc}_{i}")
                    nc.tensor.transpose(
                        tp[:], vt[:, i * 128:(i + 1) * 128], ident[:])
                    kt_ix = rc * (RC // 128) + i
                    nc.vector.tensor_copy(
                        V_sb[:, kt_ix, :, 0:32],
                        tp[:].rearrange("p (h c) -> p h c", c=32))

            wout_stage = {}

            def wout_dma(j):
                st = work.tile([128, DIM], F32, tag="wstage", bufs=3,
                               name=f"wst_{j}")
                nc.sync.dma_start(st[:], wout_d[j * 128:(j + 1) * 128, :])
                wout_stage[j] = st

            def wout_cast(j):
                nc.vector.tensor_copy(wout_bf[:, j, :], wout_stage.pop(j))

            # ---------------- phase B: attention q-block ----------------
            PV_LAG = 5

            class Window:
                """One 512-wide q-block: two independent QK->exp chains (A:
                heads 0-1, B: heads 2-3) keep ACT back-to-back; PV+den
                accumulate in one packed matmul pair per chain."""

                def __init__(self, w, b, qb):
                    self.w, self.b, self.qb = w, b, qb
                    self.qsl = qT_sb[:, b * N + qb * QB: b * N + qb * QB + QB]
                    self.pvd = [
                        ps.tile([128, QB], F32, tag=f"pvd{x}",
                                name=f"pvd{x}_{w}")
                        for x in "ab"]
                    self.exps = []

                def qk_exp(self, kt):
                    b = self.b
                    ksl = kT_sb[:, b * N + kt * 128: b * N + kt * 128 + 128]
                    es = []
                    for hp in range(2):
                        qk = ps.tile([128, 2, QB], F32, tag=f"qk{'ab'[hp]}",
                                     name=f"qk{'ab'[hp]}_{self.w}_{kt}")
                        e = work.tile([128, 2, QB], BF16, tag=f"exp{'ab'[hp]}",
                                      bufs=PV_LAG + 2,
                                      name=f"exp{'ab'[hp]}_{self.w}_{kt}")
                        for i in range(2):
                            h = 2 * hp + i
                            nc.tensor.matmul(
                                qk[:, i, :],
                                ksl[32 * h:32 * h + 32, :],
                                self.qsl[32 * h:32 * h + 32, :],
                                start=True, stop=True,
                                tile_position=(32 * h, 0))
                        nc.scalar.activation(e[:], qk[:], AF.Exp, scale=SCALE)
                        es.append(e)
                    self.exps.append(es)

                def pv(self, kt):
                    es = self.exps[kt]
                    for hp in range(2):
                        for i in range(2):
                            h = 2 * hp + i
                            nc.tensor.matmul(
                                self.pvd[hp][64 * i:64 * i + 33, :],
                                V_sb[:, self.b * KT + kt, h, :],
                                es[hp][:, i, :],
                                start=(kt == 0), stop=(kt == KT - 1),
                                tile_position=(0, 64 * i))

            def epilogue(win):
                """Normalize pvd by recip(den) broadcast over each head's 32
                channels (indicator matmul), drain to bf16, exchange."""
                w = win.w
                # den for head h lives at pvd[h//2] partition 32 + 64*(h%2)
                den4 = work.tile([128, QB], F32, tag="den4", bufs=2,
                                 name=f"den4_{w}")
                nc.vector.memset(den4[:], 1.0)
                for h in range(HPC):
                    nc.vector.tensor_copy(
                        den4[32 * h:32 * h + 1, :],
                        win.pvd[h // 2][64 * (h % 2) + 32:
                                        64 * (h % 2) + 33, :])
                recipf = work.tile([128, QB], F32, tag="recipf", bufs=2,
                                   name=f"recipf_{w}")
                nc.vector.reciprocal_approx_fast(out=recipf[:], in_=den4[:])
                recipb = work.tile([128, QB], BF16, tag="recipb", bufs=2,
                                   name=f"recipb_{w}")
                nc.vector.tensor_copy(recipb[:], recipf[:])
                rbc = ps.tile([128, QB], F32, tag="aux", name=f"rbc_{w}")
                nc.tensor.matmul(rbc[:], e4[:], recipb[:],
                                 start=True, stop=True)
                rbc_sb = work.tile([128, QB], F32, tag="rbcsb", bufs=2,
                                   name=f"rbcsb_{w}")
                nc.vector.tensor_copy(rbc_sb[:], rbc[:])
                att = work.tile([128, QB], BF16, tag="att", bufs=2,
                                name=f"att_{w}")
                for h in range(HPC):
                    nc.vector.tensor_tensor(
                        att[32 * h:32 * h + 32, :],
                        win.pvd[h // 2][64 * (h % 2):64 * (h % 2) + 32, :],
                        rbc_sb[32 * h:32 * h + 32, :], ALU.mult)
                if debug:
                    nc.sync.dma_start(dbg_att[w], att[:])
                    nc.sync.dma_start(dbg_rec[w], rbc_sb[:])
                for j in range(N_CORES):
                    nc.sync.dma_start(a2a_in[w][j],
                                      att[:, j * RPU:(j + 1) * RPU])
                nc.gpsimd.collective_compute(
                    "AllToAll", ALU.bypass,
                    replica_groups=[list(range(N_CORES))],
                    ins=[a2a_in[w].opt()], outs=[a2a_out[w].opt()])

            # ---------------- phase C: out-proj + LN per unit ----------
            asb_tiles = {}

            def outproj_load(u):
                asb = work.tile([128, N_CORES, RPU], BF16, tag="a2asb",
                                bufs=2, name=f"asb_{u}")
                for j in range(N_CORES):
                    nc.sync.dma_start(asb[:, j, :], a2a_out[u][j])
                asb_tiles[u] = asb

            def outproj_unit(u):
                asb = asb_tiles.pop(u)
                if debug:
                    nc.sync.dma_start(dbg_asb[u], asb[:])
                osb = work.tile([RPU, DIM], F32, tag="osb", bufs=2,
                                name=f"osb_{u}")
                for nb in range(2):
                    op = ps.tile([RPU, 512], F32, tag="aux",
                                 name=f"op_{u}_{nb}")
                    for j in range(N_CORES):
                        nc.tensor.matmul(
                            op[:], asb[:, j, :],
                            wout_bf[:, j, nb * 512:(nb + 1) * 512],
                            start=(j == 0), stop=(j == N_CORES - 1))
                    nc.vector.tensor_tensor(
                        osb[:, nb * 512:(nb + 1) * 512], op[:],
                        bout_bc[0:RPU, nb * 512:(nb + 1) * 512], ALU.add)
                if debug:
                    nc.sync.dma_start(dbg_osb[u], osb[:])
                # LayerNorm over the 1024 free dim
                stats = work.tile([RPU, 2, 6], F32, tag="stats", bufs=2,
                                  name=f"stats_{u}")
                for sg in range(2):
                    nc.vector.bn_stats(out=stats[:, sg, :],
                                       in_=osb[:, sg * 512:(sg + 1) * 512])
                mv = work.tile([RPU, 2], F32, tag="mv", bufs=2,
                               name=f"mv_{u}")
                nc.vector.bn_aggr(out=mv[:], in_=stats[:])
                # rstd = exp(-0.5*ln(var+eps)); Ln+Exp share one ACT table
                lnv = work.tile([RPU, 1], F32, tag="lnv", bufs=2,
                                name=f"lnv_{u}")
                nc.scalar.activation(out=lnv[:], in_=mv[:, 1:2], func=AF.Ln,
                                     bias=eps_sb[0:RPU, :], scale=1.0)
                rstd = work.tile([RPU, 1], F32, tag="rstd", bufs=2,
                                 name=f"rstd_{u}")
                nc.scalar.activation(out=rstd[:], in_=lnv[:], func=AF.Exp,
                                     bias=0.0, scale=-0.5)
                nc.vector.tensor_scalar(
                    out=osb[:], in0=osb[:], scalar1=mv[:, 0:1],
                    scalar2=rstd[:], op0=ALU.subtract, op1=ALU.mult)
                nc.vector.tensor_tensor(osb[:], osb[:], gamma_bc[0:RPU, :],
                                        ALU.mult)
                nc.vector.tensor_tensor(osb[:], osb[:], beta_bc[0:RPU, :],
                                        ALU.add)
                nc.sync.dma_start(out_d[u * RPU:(u + 1) * RPU, :], osb[:])

            # ---------------- issue order (software pipeline) ----------
            # Window w's kt loop carries: the epilogue of window w-1 (kt=2),
            # out-proj of unit w-2 (DMAs kt=6, matmuls kt=10), and q/k/v
            # projection parts + weight loads at fixed slots, with DMAs
            # prefetched well before their consumers.
            proj_dma(0)
            proj_dma(1)
            proj_part(0, "k")
            proj_part(0, "v")
            proj_part(0, "q")
            # chunk parts: (slot kt) -> (rc, part); b0 rc1-3 in window 0,
            # b1 rc4-7 over windows 1-2
            part_slots = {
                0: {2: (1, "k"), 3: (1, "v"), 4: (1, "q"),
                    6: (2, "k"), 7: (2, "v"), 8: (2, "q"),
                    10: (3, "k"), 11: (3, "v"), 12: (3, "q")},
                1: {3: (4, "k"), 5: (4, "v"), 7: (4, "q"),
                    9: (5, "k"), 11: (5, "v"), 13: (5, "q")},
                2: {3: (6, "k"), 5: (6, "v"), 7: (6, "q"),
                    9: (7, "k"), 11: (7, "v"), 13: (7, "q")},
            }
            pd_slots = {0: {0: 2, 4: 3}, 1: {0: 4, 8: 5}, 2: {0: 6, 8: 7}}
            wd_slots = {1: {1: 0, 3: 1, 5: 2, 7: 3, 9: 4, 11: 5, 13: 6,
                            15: 7}}
            wc_slots = {1: {5: 0, 7: 1, 9: 2, 11: 3, 13: 4, 15: 5},
                        2: {1: 6, 3: 7}}

            prev = None
            for w in range(NU):
                b, qb = divmod(w, NQB)
                win = Window(w, b, qb)
                for kt in range(KT):
                    win.qk_exp(kt)
                    if kt == 2 and prev is not None:
                        epilogue(prev)
                    if kt >= PV_LAG:
                        win.pv(kt - PV_LAG)
                    rc = pd_slots.get(w, {}).get(kt)
                    if rc is not None:
                        proj_dma(rc)
                    ps_ = part_slots.get(w, {}).get(kt)
                    if ps_ is not None:
                        proj_part(*ps_)
                    j = wd_slots.get(w, {}).get(kt)
                    if j is not None:
                        wout_dma(j)
                    j = wc_slots.get(w, {}).get(kt)
                    if j is not None:
                        wout_cast(j)
                    if kt == 6 and w >= 2:
                        outproj_load(w - 2)
                    if kt == 10 and w >= 2:
                        outproj_unit(w - 2)
                for kt in range(KT - PV_LAG, KT):
                    win.pv(kt)
                prev = win
            epilogue(prev)
            outproj_load(NU - 2)
            outproj_unit(NU - 2)
            outproj_load(NU - 1)
            outproj_unit(NU - 1)

            if debug:
                nc.sync.dma_start(dbg_qT, qT_sb[:].bitcast(F32))
                nc.sync.dma_start(dbg_kT, kT_sb[:].bitcast(F32))
                nc.sync.dma_start(dbg_V, V_sb[:])

    nc.compile()
    return nc


class _Runner:
    """Compile once; run the SPMD kernel on 8 cores via PJRT repeatedly."""

    def __init__(self, debug=False):
        self.nc = _build(debug=debug)
        import jax
        from jax.sharding import Mesh, NamedSharding, PartitionSpec
        from jax.experimental.shard_map import shard_map
        from concourse import bass2jax
        bass2jax.install_neuronx_cc_hook()

        nc = self.nc
        part_name = (nc.partition_id_tensor.name
                     if nc.partition_id_tensor else None)
        in_names, out_names, out_avals = [], [], []
        for alloc in nc.m.functions[0].allocations:
            if not isinstance(alloc, mybir.MemoryLocationSet):
                continue
            name = alloc.memorylocations[0].name
            if alloc.kind == "ExternalInput":
                if name != part_name:
                    in_names.append(name)
            elif alloc.kind == "ExternalOutput":
                out_names.append(name)
                out_avals.append(jax.core.ShapedArray(
                    tuple(alloc.tensor_shape), mybir.dt.np(alloc.dtype)))
        self.in_names = list(in_names)
        self.out_names = out_names
        self.out_avals = out_avals
        all_in_names = in_names + out_names
        if part_name is not None:
            all_in_names = all_in_names + [part_name]

        def _body(*args):
            operands = list(args)
            if part_name is not None:
                operands.append(bass2jax.partition_id_tensor())
            outs = bass2jax._bass_exec_p.bind(
                *operands, out_avals=tuple(out_avals),
                in_names=tuple(all_in_names), out_names=tuple(out_names),
                lowering_input_output_aliases=(),
                sim_require_finite=True, sim_require_nnan=True, nc=nc)
            return tuple(outs)

        devices = jax.devices()[:N_CORES]
        mesh = Mesh(np.asarray(devices), ("core",))
        self.sharding = NamedSharding(mesh, PartitionSpec("core"))
        nin = len(self.in_names) + len(out_names)
        self.fn = jax.jit(shard_map(
            _body, mesh=mesh, in_specs=(PartitionSpec("core"),) * nin,
            out_specs=(PartitionSpec("core"),) * len(out_names),
            check_rep=False))
        self.jax = jax

    def stage(self, in_maps):
        """Concatenate per-core inputs + zero outputs; device_put SHARDED so
        no per-call resharding happens."""
        concat = [np.concatenate([m[name] for m in in_maps], axis=0)
                  for name in self.in_names]
        zeros = [np.zeros((N_CORES * a.shape[0], *a.shape[1:]), a.dtype)
                 for a in self.out_avals]
        return [self.jax.device_put(x, self.sharding)
                for x in concat + zeros]

    def run_staged(self, staged):
        outs = self.fn(*staged)
        self.jax.block_until_ready(outs)
        return outs

    def run(self, in_maps):
        outs = self.run_staged(self.stage(in_maps))
        return [
            {name: np.asarray(outs[i]).reshape(
                N_CORES, *self.out_avals[i].shape)[c]
             for i, name in enumerate(self.out_names)}
            for c in range(N_CORES)
        ]


_RUNNER = None


def _get_runner():
    global _RUNNER
    if _RUNNER is None:
        _RUNNER = _Runner()
    return _RUNNER


def _make_in_maps(x, w_qkv, w_out, b_out, ln_gamma, ln_beta):
    x = np.asarray(x, dtype=np.float32)
    w_qkv = np.asarray(w_qkv, dtype=np.float32)
    w_out = np.asarray(w_out, dtype=np.float32)
    b_out = np.asarray(b_out, dtype=np.float32)
    ln_gamma = np.asarray(ln_gamma, dtype=np.float32)
    ln_beta = np.asarray(ln_beta, dtype=np.float32)

    xT = np.ascontiguousarray(x.reshape(ROWS, DIM).T)
    in_maps = []
    for c in range(N_CORES):
        h0 = HPC * c * DH
        cols = np.concatenate([
            w_qkv[:, h0:h0 + HPC * DH],
            w_qkv[:, DIM + h0:DIM + h0 + HPC * DH],
            w_qkv[:, 2 * DIM + h0:2 * DIM + h0 + HPC * DH],
        ], axis=1)
        in_maps.append({
            "xT": xT,
            "wqkv": np.ascontiguousarray(cols),
            "wout": w_out,
            "bout": b_out,
            "gamma": ln_gamma,
            "beta": ln_beta,
        })
    return in_maps


def kernel(x, w_qkv, w_out, b_out, ln_gamma, ln_beta):
    runner = _get_runner()
    in_maps = _make_in_maps(x, w_qkv, w_out, b_out, ln_gamma, ln_beta)
    results = runner.run(in_maps)
    # core c, unit u=(b, qb) holds rows b*2048 + qb*512 + 64*c + [0..64)
    out = np.empty((ROWS, DIM), np.float32)
    for c in range(N_CORES):
        oc = results[c]["out"]
        for u in range(NU):
            b, qb = divmod(u, NQB)
            r0 = b * N + qb * QB + RPU * c
            out[r0:r0 + RPU] = oc[u * RPU:(u + 1) * RPU]
    return out.reshape(B, N, DIM).astype(np.float32)
